# revision 16
# baseline (speedup 1.0000x reference)
"""DPConv (kernel=8, ext=4, stride=4) on 8 TRN2 NeuronCores.

Math: with K = k + 2e = 16 and k = 8, every adaptive-pool bin is exactly
2 wide, so the whole DPConv collapses to a separable linear operator:

    out_img = L @ img @ L.T          (per (n, c) image, 128x128)

The H-side L is a matmul (contraction over partitions). The W-side L
factors as fold(F) . pairsum(S); both act on the free axis and commute
with the H-side matmul. S is ONE dense DVE add per half-chunk,
pq[v] = x[2v] + x[2v+1]; F is folded INTO the matmul: PSUM accumulates
out[4+4a+b] = LQ @ pq[2a+b] + LQ @ pq[2+2a+b] via two matmuls whose rhs
APs read pq with overlapping strides (strided rhs streams at full PE
rate - HW verified). Edge columns are two extra small matmuls - the
outer pair uses a x2-scaled copy of the weights (lt2 = 2*LQ^T) so ONE
x2-scaled ACT copy per half evacuates all edge columns. GpSimd does no
compute; it serves as a third DMA descriptor path (SWDGE).

Memory regime: the gate is rel err < 2e-2 and this pipeline measures
~5e-3, so I/O is bf16 both ways (host casts in, host upcasts out),
halving HBM traffic vs fp32 to 2 MiB in + 2 MiB out per core. A single
HWDGE stream runs at ~170 GB/s, so each direction is split across two
descriptor queues (in: sync+gpsimd, out: scalar+sync) in 256 KiB
half-chunks to approach the ~358 GB/s per-core HBM limit.

Engine roles per 16-image group (two 8-image halves h):
  DVE   pq[h] = x_even + x_odd (dense bf16); evac half h1 (CAST)
  PE    8x 480-free fold matmuls (one PSUM bank each) + 2 edge matmuls
  ACT   evac half h0; x2 edge evac per half; out-DMA h0 - ACT runs ONLY
        output-side ops so its in-order queue never delays stores
  Sync  in-DMA h0, out-DMA h1;  GpSimd(SWDGE)  in-DMA h1

Sharding: pure data parallel - core k takes batch element n = k.
Host staging per core: transpose to [H, C, W] (contiguous DMA lines) +
bf16 cast; output returns [H, C, W] bf16, host upcasts + transposes.
"""

import ml_dtypes
import numpy as np

import concourse.bacc as bacc
import concourse.mybir as mybir
import concourse.tile as tile
from concourse import bass_utils
from concourse.ap import AP

N_CORES = 8
C_PER_CORE = 64          # images per core (= C; one batch element per core)
G = 16                   # images per compute group
H8 = 8                   # images per DMA half-chunk
N_GROUPS = C_PER_CORE // G
F32 = mybir.dt.float32
BF16 = mybir.dt.bfloat16
BF16_NP = ml_dtypes.bfloat16


def _build_lq() -> np.ndarray:
    """The 1-D DPConv operator with both 1/4 scalings folded in: L/4."""
    L = np.zeros((128, 128), np.float64)
    for w in range(128):
        i_lo = max(0, -((7 - w) // 4))      # ceil((w-7)/4)
        i_hi = min(30, w // 4)
        for i in (i_lo, i_hi):              # counted twice when equal
            L[w, min(127, max(0, 2 * w - 4 * i - 4))] += 0.25
            L[w, min(127, max(0, 2 * w - 4 * i - 3))] += 0.25
    return (L / 4.0).astype(np.float32)


_LQ_T = np.ascontiguousarray(_build_lq().T)          # lhsT layout [r, h]
_LQ_T_BF16 = _LQ_T.astype(BF16_NP)
assert np.all(_LQ_T_BF16.astype(np.float32) == _LQ_T)  # L exact in bf16
_LQ_T2_BF16 = (2.0 * _LQ_T).astype(BF16_NP)            # n/8 - still exact
assert np.all(_LQ_T2_BF16.astype(np.float32) == 2.0 * _LQ_T)


def _as_strided(base: AP, dims) -> AP:
    """Rebuild `base` (a sliced AP pointing at the wanted offset) with
    explicit [stride, size] free dims (overlapping reads allowed)."""
    return AP(base.tensor, base.offset, dims)


def _dpconv_tile(tc, o_d, x_d, lt_d, lt2_d):
    nc = tc.nc
    with tc.tile_pool(name="const", bufs=1) as cp, \
         tc.tile_pool(name="in", bufs=8) as inp, \
         tc.tile_pool(name="io", bufs=3) as iop, \
         tc.tile_pool(name="mid", bufs=3) as mp, \
         tc.tile_pool(name="pm", bufs=3, space="PSUM") as pmp, \
         tc.tile_pool(name="pe", bufs=1, space="PSUM") as pep, \
         tc.tile_pool(name="pw", bufs=1, space="PSUM") as pwp:
        lt = cp.tile([128, 128], BF16, tag="lt")
        nc.sync.dma_start(out=lt[:], in_=lt_d)
        lt2 = cp.tile([128, 128], BF16, tag="lt2")
        nc.sync.dma_start(out=lt2[:], in_=lt2_d)
        # PE HAM warm-up: ~3.4 us of dummy matmuls while the first input
        # chunk is in flight, so real matmuls run at 2.4 GHz from group 0
        wp = pwp.tile([128, 128], F32, tag="warm")
        for _ in range(28):
            nc.tensor.matmul(wp[:], lt[:], lt[:], start=True, stop=True)
        for g in range(N_GROUPS):
            # one 512 KiB DMA per group (c-block-major DRAM layout gives
            # 4 KiB contiguous per-partition descriptor runs). Early
            # groups ride the SWDGE queue - it wins the SDMA round-robin
            # against HWDGE, so g0/g1 land first and the pipeline starts
            # ~3 us sooner; sync (starved until SWDGE drains) carries
            # g2/g3 whose data is needed later.
            ct = inp.tile([128, G, 128], BF16, tag="in")
            if g == 0:
                # group 0 split across BOTH queues so the pipeline's
                # first data lands ~1.2 us earlier
                nc.gpsimd.dma_start(out=ct[:, 0:H8, :], in_=x_d[0, :, 0:H8, :])
                nc.sync.dma_start(out=ct[:, H8:G, :], in_=x_d[0, :, H8:G, :])
            else:
                eng = nc.gpsimd if g == 1 else nc.sync
                eng.dma_start(out=ct[:], in_=x_d[g])

            # pq[v] = x[2v] + x[2v+1]: dense bf16 adds, split DVE (10c) /
            # GpSimd (6c) so DVE's evac CAST never queues behind pairsums
            pq = mp.tile([128, G, 64], BF16, tag="pq")
            qdim = list(pq[:].ap[1])            # [64, G]
            pdim0 = list(pq[:].ap[0])           # partition dim
            nc.vector.tensor_add(
                out=pq[:, 0:10, :], in0=ct[:, 0:10, 0:128:2],
                in1=ct[:, 0:10, 1:128:2])
            nc.gpsimd.tensor_add(
                out=pq[:, 10:G, :], in0=ct[:, 10:G, 0:128:2],
                in1=ct[:, 10:G, 1:128:2])

            # fold via PSUM accumulation, per 4-image c-chunk (each
            # matmul writes exactly one 2 KiB PSUM bank):
            #   cols 4+4a+b = LQ@pq[2a+b] + LQ@pq[2+2a+b]
            ths = []
            for h in range(2):
                th = pmp.tile([128, H8, 128], F32, tag="th")
                tdim = th[:].ap
                for q in range(2):
                    cq = slice(4 * q, 4 * (q + 1))
                    out_main = _as_strided(
                        th[:, cq, 4:5],
                        [list(tdim[0]), [128, 4], [4, 30], [1, 4]])
                    cg = 4 * q + H8 * h
                    for j in range(2):
                        rhs = _as_strided(
                            pq[:, cg:cg + 1, 2 * j:2 * j + 1],
                            [pdim0, [64, 4], [2, 30], [1, 4]])
                        nc.tensor.matmul(out_main, lt[:], rhs,
                                         start=(j == 0), stop=(j == 1))
                ths.append(th)
            # edge columns into a 1-bank PSUM tile te[c, 8] =
            # [P0,P1,P2,P3,P64,P65,P66,P67] (68-col P indexing):
            #   slots {2..5} = LQ @ pq[{0,1,62,63}]      (inner, lt)
            #   slots {0,1,6,7} = 2*LQ @ x[{0,0,127,127}] (outer, lt2)
            te = pep.tile([128, G, 8], F32, tag="te")
            edim = te[:].ap
            nc.tensor.matmul(
                _as_strided(te[:, :, 2:3], [list(edim[0]), [8, G], [1, 4]]),
                lt[:],
                _as_strided(pq[:, :, 0:1], [pdim0, qdim, [62, 2], [1, 2]]),
                start=True, stop=True)
            nc.tensor.matmul(
                _as_strided(te[:, :, 0:1],
                            [list(edim[0]), [8, G], [6, 2], [1, 2]]),
                lt2[:],
                _as_strided(ct[:, :, 0:1],
                            [list(ct[:].ap[0]), [128, G], [127, 2], [0, 2]]),
                start=True, stop=True)

            # evacuate PSUM -> bf16 out tile: main cols on ACT (h0) /
            # DVE (h1), all edge cols in ONE x2 ACT copy per half
            ot = iop.tile([128, G, 128], BF16, tag="out")
            odim = ot[:].ap
            nc.scalar.copy(out=ot[:, 0:H8, 4:124], in_=ths[0][:, :, 4:124])
            nc.vector.tensor_copy(out=ot[:, H8:G, 4:124],
                                  in_=ths[1][:, :, 4:124])
            nc.scalar.mul(
                _as_strided(ot[:, :, 0:1],
                            [list(odim[0]), list(odim[1]), [124, 2], [1, 4]]),
                _as_strided(te[:, :, 0:1],
                            [list(edim[0]), [8, G], [4, 2], [1, 4]]),
                2.0)

            # stores: one 512 KiB DMA per group, alternating the SWDGE
            # queue (done with inputs by now) and the ACT ring so two
            # store streams run; sync still carries late input chunks
            eng = nc.gpsimd if g % 2 == 0 else nc.scalar
            eng.dma_start(out=o_d[g], in_=ot[:])


_CACHE = {}


def _get_nc():
    if "nc" not in _CACHE:
        nc = bacc.Bacc("TRN2", target_bir_lowering=False, debug=False)
        x_d = nc.dram_tensor("x", (N_GROUPS, 128, G, 128), BF16,
                             kind="ExternalInput").ap()
        lt_d = nc.dram_tensor("lt", (128, 128), BF16,
                              kind="ExternalInput").ap()
        lt2_d = nc.dram_tensor("lt2", (128, 128), BF16,
                               kind="ExternalInput").ap()
        o_d = nc.dram_tensor("o", (N_GROUPS, 128, G, 128), BF16,
                             kind="ExternalOutput").ap()
        with tile.TileContext(nc) as tc:
            _dpconv_tile(tc, o_d, x_d, lt_d, lt2_d)
        nc.compile()
        _CACHE["nc"] = nc
    return _CACHE["nc"]


def _stage(xk: np.ndarray) -> np.ndarray:
    """[C,H,W] f32 -> [NG,H,G,W] bf16 c-block-major: each 16-image group
    is one contiguous DRAM block with 4 KiB per-partition runs."""
    xt = xk.transpose(1, 0, 2)                       # [H, C, W]
    xb = xt.reshape(128, N_GROUPS, G, 128).transpose(1, 0, 2, 3)
    return np.ascontiguousarray(xb).astype(BF16_NP)


def run(x: np.ndarray, **spmd_kwargs) -> bass_utils.BassKernelResults:
    """Shard x (8,64,128,128) across 8 cores and run the Bass kernel."""
    nc = _get_nc()
    in_maps = [
        {"x": _stage(x[k]), "lt": _LQ_T_BF16, "lt2": _LQ_T2_BF16}
        for k in range(N_CORES)
    ]
    return bass_utils.run_bass_kernel_spmd(
        nc, in_maps, core_ids=list(range(N_CORES)), **spmd_kwargs)


def kernel(x) -> np.ndarray:
    x = np.asarray(x, dtype=np.float32)
    assert x.shape == (N_CORES, C_PER_CORE, 128, 128), x.shape
    res = run(x)
    outs = []
    for k in range(N_CORES):
        ob = res.results[k]["o"].astype(np.float32)   # [NG, H, G, W]
        oc = ob.transpose(1, 0, 2, 3).reshape(128, C_PER_CORE, 128)
        outs.append(oc.transpose(1, 0, 2))            # [C, H, W]
    return np.stack(outs, axis=0)


# revision 17
# speedup vs baseline: 1.0472x; 1.0472x over previous
"""DPConv (kernel=8, ext=4, stride=4) on 8 TRN2 NeuronCores.

Math: with K = k + 2e = 16 and k = 8, every adaptive-pool bin is exactly
2 wide, so the whole DPConv collapses to a separable linear operator:

    out_img = L @ img @ L.T          (per (n, c) image, 128x128)

The H-side L is a matmul (contraction over partitions). The W-side L
factors as fold(F) . pairsum(S); both act on the free axis and commute
with the H-side matmul. S is ONE dense DVE add per half-chunk,
pq[v] = x[2v] + x[2v+1]; F is folded INTO the matmul: PSUM accumulates
out[4+4a+b] = LQ @ pq[2a+b] + LQ @ pq[2+2a+b] via two matmuls whose rhs
APs read pq with overlapping strides (strided rhs streams at full PE
rate - HW verified). Edge columns are two extra small matmuls - the
outer pair uses a x2-scaled copy of the weights (lt2 = 2*LQ^T) so ONE
x2-scaled ACT copy per half evacuates all edge columns. GpSimd does no
compute; it serves as a third DMA descriptor path (SWDGE).

Memory regime: the gate is rel err < 2e-2 and this pipeline measures
~5e-3, so I/O is bf16 both ways (host casts in, host upcasts out),
halving HBM traffic vs fp32 to 2 MiB in + 2 MiB out per core. A single
HWDGE stream runs at ~170 GB/s, so each direction is split across two
descriptor queues (in: sync+gpsimd, out: scalar+sync) in 256 KiB
half-chunks to approach the ~358 GB/s per-core HBM limit.

Engine roles per 16-image group (two 8-image halves h):
  DVE   pq[h] = x_even + x_odd (dense bf16); evac half h1 (CAST)
  PE    8x 480-free fold matmuls (one PSUM bank each) + 2 edge matmuls
  ACT   evac half h0; x2 edge evac per half; out-DMA h0 - ACT runs ONLY
        output-side ops so its in-order queue never delays stores
  Sync  in-DMA h0, out-DMA h1;  GpSimd(SWDGE)  in-DMA h1

Sharding: pure data parallel - core k takes batch element n = k.
Host staging per core: transpose to [H, C, W] (contiguous DMA lines) +
bf16 cast; output returns [H, C, W] bf16, host upcasts + transposes.
"""

import ml_dtypes
import numpy as np

import concourse.bacc as bacc
import concourse.mybir as mybir
import concourse.tile as tile
from concourse import bass_utils
from concourse.ap import AP

N_CORES = 8
C_PER_CORE = 64          # images per core (= C; one batch element per core)
G = 16                   # images per compute group
H8 = 8                   # images per DMA half-chunk
N_GROUPS = C_PER_CORE // G
F32 = mybir.dt.float32
BF16 = mybir.dt.bfloat16
BF16_NP = ml_dtypes.bfloat16


def _build_lq() -> np.ndarray:
    """The 1-D DPConv operator with both 1/4 scalings folded in: L/4."""
    L = np.zeros((128, 128), np.float64)
    for w in range(128):
        i_lo = max(0, -((7 - w) // 4))      # ceil((w-7)/4)
        i_hi = min(30, w // 4)
        for i in (i_lo, i_hi):              # counted twice when equal
            L[w, min(127, max(0, 2 * w - 4 * i - 4))] += 0.25
            L[w, min(127, max(0, 2 * w - 4 * i - 3))] += 0.25
    return (L / 4.0).astype(np.float32)


_LQ_T = np.ascontiguousarray(_build_lq().T)          # lhsT layout [r, h]
_LQ_T_BF16 = _LQ_T.astype(BF16_NP)
assert np.all(_LQ_T_BF16.astype(np.float32) == _LQ_T)  # L exact in bf16
_LQ_T2_BF16 = (2.0 * _LQ_T).astype(BF16_NP)            # n/8 - still exact
assert np.all(_LQ_T2_BF16.astype(np.float32) == 2.0 * _LQ_T)


def _as_strided(base: AP, dims) -> AP:
    """Rebuild `base` (a sliced AP pointing at the wanted offset) with
    explicit [stride, size] free dims (overlapping reads allowed)."""
    return AP(base.tensor, base.offset, dims)


def _dpconv_tile(tc, o_d, x_d, lt_d, lt2_d):
    nc = tc.nc
    with tc.tile_pool(name="const", bufs=1) as cp, \
         tc.tile_pool(name="in", bufs=8) as inp, \
         tc.tile_pool(name="io", bufs=3) as iop, \
         tc.tile_pool(name="mid", bufs=3) as mp, \
         tc.tile_pool(name="pm", bufs=3, space="PSUM") as pmp, \
         tc.tile_pool(name="pe", bufs=1, space="PSUM") as pep, \
         tc.tile_pool(name="pw", bufs=1, space="PSUM") as pwp:
        lt = cp.tile([128, 128], BF16, tag="lt")
        nc.sync.dma_start(out=lt[:], in_=lt_d)
        lt2 = cp.tile([128, 128], BF16, tag="lt2")
        nc.sync.dma_start(out=lt2[:], in_=lt2_d)
        # PE HAM warm-up: ~3.4 us of dummy matmuls while the first input
        # chunk is in flight, so real matmuls run at 2.4 GHz from group 0
        wp = pwp.tile([128, 128], F32, tag="warm")
        for _ in range(28):
            nc.tensor.matmul(wp[:], lt[:], lt[:], start=True, stop=True)
        for g in range(N_GROUPS):
            # one 512 KiB DMA per group (c-block-major DRAM layout gives
            # 4 KiB contiguous per-partition descriptor runs). Early
            # groups ride the SWDGE queue - it wins the SDMA round-robin
            # against HWDGE, so g0/g1 land first and the pipeline starts
            # ~3 us sooner; sync (starved until SWDGE drains) carries
            # g2/g3 whose data is needed later.
            ct = inp.tile([128, G, 128], BF16, tag="in")
            if g == 0:
                # group 0 split across BOTH queues so the pipeline's
                # first data lands ~1.2 us earlier
                nc.gpsimd.dma_start(out=ct[:, 0:H8, :], in_=x_d[0, :, 0:H8, :])
                nc.sync.dma_start(out=ct[:, H8:G, :], in_=x_d[0, :, H8:G, :])
            else:
                eng = nc.gpsimd if g == 1 else nc.sync
                eng.dma_start(out=ct[:], in_=x_d[g])

            # pq[v] = x[2v] + x[2v+1]: one dense bf16 DVE add
            pq = mp.tile([128, G, 64], BF16, tag="pq")
            qdim = list(pq[:].ap[1])            # [64, G]
            pdim0 = list(pq[:].ap[0])           # partition dim
            nc.vector.tensor_add(
                out=pq[:], in0=ct[:, :, 0:128:2], in1=ct[:, :, 1:128:2])

            # fold via PSUM accumulation, per 4-image c-chunk (each
            # matmul writes exactly one 2 KiB PSUM bank):
            #   cols 4+4a+b = LQ@pq[2a+b] + LQ@pq[2+2a+b]
            ths = []
            for h in range(2):
                th = pmp.tile([128, H8, 128], F32, tag="th")
                tdim = th[:].ap
                for q in range(2):
                    cq = slice(4 * q, 4 * (q + 1))
                    out_main = _as_strided(
                        th[:, cq, 4:5],
                        [list(tdim[0]), [128, 4], [4, 30], [1, 4]])
                    cg = 4 * q + H8 * h
                    for j in range(2):
                        rhs = _as_strided(
                            pq[:, cg:cg + 1, 2 * j:2 * j + 1],
                            [pdim0, [64, 4], [2, 30], [1, 4]])
                        nc.tensor.matmul(out_main, lt[:], rhs,
                                         start=(j == 0), stop=(j == 1))
                ths.append(th)
            # edge columns into a 1-bank PSUM tile te[c, 8] =
            # [P0,P1,P2,P3,P64,P65,P66,P67] (68-col P indexing):
            #   slots {2..5} = LQ @ pq[{0,1,62,63}]      (inner, lt)
            #   slots {0,1,6,7} = 2*LQ @ x[{0,0,127,127}] (outer, lt2)
            te = pep.tile([128, G, 8], F32, tag="te")
            edim = te[:].ap
            nc.tensor.matmul(
                _as_strided(te[:, :, 2:3], [list(edim[0]), [8, G], [1, 4]]),
                lt[:],
                _as_strided(pq[:, :, 0:1], [pdim0, qdim, [62, 2], [1, 2]]),
                start=True, stop=True)
            nc.tensor.matmul(
                _as_strided(te[:, :, 0:1],
                            [list(edim[0]), [8, G], [6, 2], [1, 2]]),
                lt2[:],
                _as_strided(ct[:, :, 0:1],
                            [list(ct[:].ap[0]), [128, G], [127, 2], [0, 2]]),
                start=True, stop=True)

            # evacuate PSUM -> bf16 out tile: main cols on ACT (h0) /
            # DVE (h1), all edge cols in ONE x2 ACT copy per half
            ot = iop.tile([128, G, 128], BF16, tag="out")
            odim = ot[:].ap
            nc.scalar.copy(out=ot[:, 0:H8, 4:124], in_=ths[0][:, :, 4:124])
            nc.vector.tensor_copy(out=ot[:, H8:G, 4:124],
                                  in_=ths[1][:, :, 4:124])
            nc.scalar.mul(
                _as_strided(ot[:, :, 0:1],
                            [list(odim[0]), list(odim[1]), [124, 2], [1, 4]]),
                _as_strided(te[:, :, 0:1],
                            [list(edim[0]), [8, G], [4, 2], [1, 4]]),
                2.0)

            # stores: one 512 KiB DMA per group, alternating the SWDGE
            # queue (done with inputs by now) and the ACT ring so two
            # store streams run; sync still carries late input chunks
            eng = nc.gpsimd if g % 2 == 0 else nc.scalar
            eng.dma_start(out=o_d[g], in_=ot[:])


_CACHE = {}


def _get_nc():
    if "nc" not in _CACHE:
        nc = bacc.Bacc("TRN2", target_bir_lowering=False, debug=False)
        x_d = nc.dram_tensor("x", (N_GROUPS, 128, G, 128), BF16,
                             kind="ExternalInput").ap()
        lt_d = nc.dram_tensor("lt", (128, 128), BF16,
                              kind="ExternalInput").ap()
        lt2_d = nc.dram_tensor("lt2", (128, 128), BF16,
                               kind="ExternalInput").ap()
        o_d = nc.dram_tensor("o", (N_GROUPS, 128, G, 128), BF16,
                             kind="ExternalOutput").ap()
        with tile.TileContext(nc) as tc:
            _dpconv_tile(tc, o_d, x_d, lt_d, lt2_d)
        nc.compile()
        _CACHE["nc"] = nc
    return _CACHE["nc"]


def _stage(xk: np.ndarray) -> np.ndarray:
    """[C,H,W] f32 -> [NG,H,G,W] bf16 c-block-major: each 16-image group
    is one contiguous DRAM block with 4 KiB per-partition runs."""
    xt = xk.transpose(1, 0, 2)                       # [H, C, W]
    xb = xt.reshape(128, N_GROUPS, G, 128).transpose(1, 0, 2, 3)
    return np.ascontiguousarray(xb).astype(BF16_NP)


def run(x: np.ndarray, **spmd_kwargs) -> bass_utils.BassKernelResults:
    """Shard x (8,64,128,128) across 8 cores and run the Bass kernel."""
    nc = _get_nc()
    in_maps = [
        {"x": _stage(x[k]), "lt": _LQ_T_BF16, "lt2": _LQ_T2_BF16}
        for k in range(N_CORES)
    ]
    return bass_utils.run_bass_kernel_spmd(
        nc, in_maps, core_ids=list(range(N_CORES)), **spmd_kwargs)


def kernel(x) -> np.ndarray:
    x = np.asarray(x, dtype=np.float32)
    assert x.shape == (N_CORES, C_PER_CORE, 128, 128), x.shape
    res = run(x)
    outs = []
    for k in range(N_CORES):
        ob = res.results[k]["o"].astype(np.float32)   # [NG, H, G, W]
        oc = ob.transpose(1, 0, 2, 3).reshape(128, C_PER_CORE, 128)
        outs.append(oc.transpose(1, 0, 2))            # [C, H, W]
    return np.stack(outs, axis=0)


# revision 18
# speedup vs baseline: 1.0639x; 1.0160x over previous
"""DPConv (kernel=8, ext=4, stride=4) on 8 TRN2 NeuronCores.

Math: with K = k + 2e = 16 and k = 8, every adaptive-pool bin is exactly
2 wide, so the whole DPConv collapses to a separable linear operator:

    out_img = L @ img @ L.T          (per (n, c) image, 128x128)

The H-side L is a matmul (contraction over partitions). The W-side L
factors as fold(F) . pairsum(S); both act on the free axis and commute
with the H-side matmul. S is ONE dense DVE add per half-chunk,
pq[v] = x[2v] + x[2v+1]; F is folded INTO the matmul: PSUM accumulates
out[4+4a+b] = LQ @ pq[2a+b] + LQ @ pq[2+2a+b] via two matmuls whose rhs
APs read pq with overlapping strides (strided rhs streams at full PE
rate - HW verified). Edge columns are two extra small matmuls - the
outer pair uses a x2-scaled copy of the weights (lt2 = 2*LQ^T) so ONE
x2-scaled ACT copy per half evacuates all edge columns. GpSimd does no
compute; it serves as a third DMA descriptor path (SWDGE).

Memory regime: the gate is rel err < 2e-2 and this pipeline measures
~5e-3, so I/O is bf16 both ways (host casts in, host upcasts out),
halving HBM traffic vs fp32 to 2 MiB in + 2 MiB out per core. A single
HWDGE stream runs at ~170 GB/s, so each direction is split across two
descriptor queues (in: sync+gpsimd, out: scalar+sync) in 256 KiB
half-chunks to approach the ~358 GB/s per-core HBM limit.

Engine roles per 16-image group (two 8-image halves h):
  DVE   pq[h] = x_even + x_odd (dense bf16); evac half h1 (CAST)
  PE    8x 480-free fold matmuls (one PSUM bank each) + 2 edge matmuls
  ACT   evac half h0; x2 edge evac per half; out-DMA h0 - ACT runs ONLY
        output-side ops so its in-order queue never delays stores
  Sync  in-DMA h0, out-DMA h1;  GpSimd(SWDGE)  in-DMA h1

Sharding: pure data parallel - core k takes batch element n = k.
Host staging per core: transpose to [H, C, W] (contiguous DMA lines) +
bf16 cast; output returns [H, C, W] bf16, host upcasts + transposes.
"""

import ml_dtypes
import numpy as np

import concourse.bacc as bacc
import concourse.mybir as mybir
import concourse.tile as tile
from concourse import bass_utils
from concourse.ap import AP

N_CORES = 8
C_PER_CORE = 64          # images per core (= C; one batch element per core)
G = 16                   # images per compute group
H8 = 8                   # images per DMA half-chunk
N_GROUPS = C_PER_CORE // G
F32 = mybir.dt.float32
BF16 = mybir.dt.bfloat16
BF16_NP = ml_dtypes.bfloat16


def _build_lq() -> np.ndarray:
    """The 1-D DPConv operator with both 1/4 scalings folded in: L/4."""
    L = np.zeros((128, 128), np.float64)
    for w in range(128):
        i_lo = max(0, -((7 - w) // 4))      # ceil((w-7)/4)
        i_hi = min(30, w // 4)
        for i in (i_lo, i_hi):              # counted twice when equal
            L[w, min(127, max(0, 2 * w - 4 * i - 4))] += 0.25
            L[w, min(127, max(0, 2 * w - 4 * i - 3))] += 0.25
    return (L / 4.0).astype(np.float32)


_LQ_T = np.ascontiguousarray(_build_lq().T)          # lhsT layout [r, h]
_LQ_T_BF16 = _LQ_T.astype(BF16_NP)
assert np.all(_LQ_T_BF16.astype(np.float32) == _LQ_T)  # L exact in bf16
_LQ_T2_BF16 = (2.0 * _LQ_T).astype(BF16_NP)            # n/8 - still exact
assert np.all(_LQ_T2_BF16.astype(np.float32) == 2.0 * _LQ_T)


def _as_strided(base: AP, dims) -> AP:
    """Rebuild `base` (a sliced AP pointing at the wanted offset) with
    explicit [stride, size] free dims (overlapping reads allowed)."""
    return AP(base.tensor, base.offset, dims)


def _dpconv_tile(tc, o_d, x_d, lt_d, lt2_d):
    nc = tc.nc
    with tc.tile_pool(name="const", bufs=1) as cp, \
         tc.tile_pool(name="in", bufs=8) as inp, \
         tc.tile_pool(name="io", bufs=3) as iop, \
         tc.tile_pool(name="mid", bufs=3) as mp, \
         tc.tile_pool(name="pm", bufs=3, space="PSUM") as pmp, \
         tc.tile_pool(name="pe", bufs=1, space="PSUM") as pep, \
         tc.tile_pool(name="pw", bufs=1, space="PSUM") as pwp:
        lt = cp.tile([128, 128], BF16, tag="lt")
        nc.sync.dma_start(out=lt[:], in_=lt_d)
        lt2 = cp.tile([128, 128], BF16, tag="lt2")
        nc.sync.dma_start(out=lt2[:], in_=lt2_d)
        # PE HAM warm-up: ~3.4 us of dummy matmuls while the first input
        # chunk is in flight, so real matmuls run at 2.4 GHz from group 0
        wp = pwp.tile([128, 128], F32, tag="warm")
        for _ in range(28):
            nc.tensor.matmul(wp[:], lt[:], lt[:], start=True, stop=True)
        for g in range(N_GROUPS):
            # one 512 KiB DMA per group (c-block-major DRAM layout gives
            # 4 KiB contiguous per-partition descriptor runs). Early
            # groups ride the SWDGE queue - it wins the SDMA round-robin
            # against HWDGE, so g0/g1 land first and the pipeline starts
            # ~3 us sooner; sync (starved until SWDGE drains) carries
            # g2/g3 whose data is needed later.
            ct = inp.tile([128, G, 128], BF16, tag="in")
            eng = nc.gpsimd if g < 2 else nc.sync
            eng.dma_start(out=ct[:], in_=x_d[g])

            # pq[v] = x[2v] + x[2v+1]: one dense bf16 DVE add
            pq = mp.tile([128, G, 64], BF16, tag="pq")
            qdim = list(pq[:].ap[1])            # [64, G]
            pdim0 = list(pq[:].ap[0])           # partition dim
            nc.vector.tensor_add(
                out=pq[:], in0=ct[:, :, 0:128:2], in1=ct[:, :, 1:128:2])

            # fold via PSUM accumulation, per 4-image c-chunk (each
            # matmul writes exactly one 2 KiB PSUM bank):
            #   cols 4+4a+b = LQ@pq[2a+b] + LQ@pq[2+2a+b]
            ths = []
            for h in range(2):
                th = pmp.tile([128, H8, 128], F32, tag="th")
                tdim = th[:].ap
                for q in range(2):
                    cq = slice(4 * q, 4 * (q + 1))
                    out_main = _as_strided(
                        th[:, cq, 4:5],
                        [list(tdim[0]), [128, 4], [4, 30], [1, 4]])
                    cg = 4 * q + H8 * h
                    for j in range(2):
                        rhs = _as_strided(
                            pq[:, cg:cg + 1, 2 * j:2 * j + 1],
                            [pdim0, [64, 4], [2, 30], [1, 4]])
                        nc.tensor.matmul(out_main, lt[:], rhs,
                                         start=(j == 0), stop=(j == 1))
                ths.append(th)
            # edge columns into a 1-bank PSUM tile te[c, 8] =
            # [P0,P1,P2,P3,P64,P65,P66,P67] (68-col P indexing):
            #   slots {2..5} = LQ @ pq[{0,1,62,63}]      (inner, lt)
            #   slots {0,1,6,7} = 2*LQ @ x[{0,0,127,127}] (outer, lt2)
            te = pep.tile([128, G, 8], F32, tag="te")
            edim = te[:].ap
            nc.tensor.matmul(
                _as_strided(te[:, :, 2:3], [list(edim[0]), [8, G], [1, 4]]),
                lt[:],
                _as_strided(pq[:, :, 0:1], [pdim0, qdim, [62, 2], [1, 2]]),
                start=True, stop=True)
            nc.tensor.matmul(
                _as_strided(te[:, :, 0:1],
                            [list(edim[0]), [8, G], [6, 2], [1, 2]]),
                lt2[:],
                _as_strided(ct[:, :, 0:1],
                            [list(ct[:].ap[0]), [128, G], [127, 2], [0, 2]]),
                start=True, stop=True)

            # evacuate PSUM -> bf16 out tile: main cols on ACT (h0) /
            # DVE (h1), all edge cols in ONE x2 ACT copy per half
            ot = iop.tile([128, G, 128], BF16, tag="out")
            odim = ot[:].ap
            nc.scalar.copy(out=ot[:, 0:H8, 4:124], in_=ths[0][:, :, 4:124])
            nc.vector.tensor_copy(out=ot[:, H8:G, 4:124],
                                  in_=ths[1][:, :, 4:124])
            nc.scalar.mul(
                _as_strided(ot[:, :, 0:1],
                            [list(odim[0]), list(odim[1]), [124, 2], [1, 4]]),
                _as_strided(te[:, :, 0:1],
                            [list(edim[0]), [8, G], [4, 2], [1, 4]]),
                2.0)

            # stores: one 512 KiB DMA per group, alternating the SWDGE
            # queue (done with inputs by now) and the ACT ring so two
            # store streams run; sync still carries late input chunks
            eng = nc.gpsimd if g % 2 == 0 else nc.scalar
            eng.dma_start(out=o_d[g], in_=ot[:])


_CACHE = {}


def _get_nc():
    if "nc" not in _CACHE:
        nc = bacc.Bacc("TRN2", target_bir_lowering=False, debug=False)
        x_d = nc.dram_tensor("x", (N_GROUPS, 128, G, 128), BF16,
                             kind="ExternalInput").ap()
        lt_d = nc.dram_tensor("lt", (128, 128), BF16,
                              kind="ExternalInput").ap()
        lt2_d = nc.dram_tensor("lt2", (128, 128), BF16,
                               kind="ExternalInput").ap()
        o_d = nc.dram_tensor("o", (N_GROUPS, 128, G, 128), BF16,
                             kind="ExternalOutput").ap()
        with tile.TileContext(nc) as tc:
            _dpconv_tile(tc, o_d, x_d, lt_d, lt2_d)
        nc.compile()
        _CACHE["nc"] = nc
    return _CACHE["nc"]


def _stage(xk: np.ndarray) -> np.ndarray:
    """[C,H,W] f32 -> [NG,H,G,W] bf16 c-block-major: each 16-image group
    is one contiguous DRAM block with 4 KiB per-partition runs."""
    xt = xk.transpose(1, 0, 2)                       # [H, C, W]
    xb = xt.reshape(128, N_GROUPS, G, 128).transpose(1, 0, 2, 3)
    return np.ascontiguousarray(xb).astype(BF16_NP)


def run(x: np.ndarray, **spmd_kwargs) -> bass_utils.BassKernelResults:
    """Shard x (8,64,128,128) across 8 cores and run the Bass kernel."""
    nc = _get_nc()
    in_maps = [
        {"x": _stage(x[k]), "lt": _LQ_T_BF16, "lt2": _LQ_T2_BF16}
        for k in range(N_CORES)
    ]
    return bass_utils.run_bass_kernel_spmd(
        nc, in_maps, core_ids=list(range(N_CORES)), **spmd_kwargs)


def kernel(x) -> np.ndarray:
    x = np.asarray(x, dtype=np.float32)
    assert x.shape == (N_CORES, C_PER_CORE, 128, 128), x.shape
    res = run(x)
    outs = []
    for k in range(N_CORES):
        ob = res.results[k]["o"].astype(np.float32)   # [NG, H, G, W]
        oc = ob.transpose(1, 0, 2, 3).reshape(128, C_PER_CORE, 128)
        outs.append(oc.transpose(1, 0, 2))            # [C, H, W]
    return np.stack(outs, axis=0)
